# revision 9
# baseline (speedup 1.0000x reference)
"""Trainium2 Bass kernel for the sparse_attention PoC block.

Reference computation (per batch item):
  qkv = x @ qkv_w.T            [N, 3C] -> q,k,v heads [H, N, D]
  attn = (q @ k.T) * scale     [H, N, N]
  block edits: attn[:S1, S2:] = attn[:S1, S1:S2] (pre-bias copy), then
  -100 bias on [:S1, S1:S2], [S1:S2, S2:], [S2:, S1:S2]; softmax;
  attn @ v; proj.

Distribution: pure data-parallel over batch B=64 across 8 NeuronCores
(8 batch items per core, weights replicated). No collectives.

v3: software-pipelined emission. The PE (TensorMatrix) is the
bottleneck (~400k row-cycles ~= 167us at the 2.4 GHz max p-state); the
baseline lost ~2x to dependency stalls in the attention phase (PE idle
gaps also drop the DVFS p-state to 1.2 GHz). Batch b's attention+proj
pieces are interleaved at EMISSION level with batch b+1's
qkv-projection pieces so the in-order PE queue always has independent
matmuls between dependent ones; weight prep is likewise pipelined with
batch 0's qkv. Engine assignment: ScalarE+VectorE alternate on psum
evacuations and casts; GpSimd only does the reciprocal partition
broadcasts (it cannot read PSUM and its copy rate is ~6x worse than
V/S); per-batch q/k bias-extension rows are copied from pre-broadcast
wbig/ubig tiles (a stride-0 broadcast read costs 3us vs 0.3us
contiguous). Transposes are paired/tripled per psum tile to halve
evacuation instruction count. proj_b rides the proj matmul as a 97th
contraction row on head 7 (aoT row 96 = ones).

Layout per core (TensorE matmuls in bf16, fp32 PSUM accum):
  - x transposed on TensorE to xT [C, N]
  - q,k per head transposed: q_all/k_all [128, H, N] (rows 0:96 head
    data, 96:128 bias-extension rows); v natural [n, h, D+1] ([v|ones])
  - scores sT[j,q] = k_ext.T @ q_ext in one matmul per key tile; the
    -100 block edits ride as rank-2 updates in contraction rows 96/97;
    the pre-bias "copy" edit = overwriting kT aux-slot columns with
    lang key vectors + a 20x20 correction matmul for the aux x aux
    block
  - softmax without max-subtraction (logits O(1); suppressed entries
    underflow exp to ~0); exp on ScalarE with 1/sqrt(D) folded in
  - attn@v gives oT [D+1, q] + denominator row; normalize via approx
    reciprocal + gpsimd partition_broadcast (96 rows); the aoT mult is
    emitted 2 schedule slots after its broadcast so the in-order
    Vector queue never waits on GpSimd
  - proj psum[n, oc] = aoT.T @ proj_w^T (K=96/97 per head) -> osb ->
    DRAM (PSUM cannot be a DMA source)

Known HW constraint: reciprocal_approx_fast with a PSUM source dies at
runtime (isolated probe passes, in-kernel use does not) -- the
denominator is staged through SBUF first.

Partition-alignment rule (walrus verifier): compute-engine access
patterns must start at partition 0/32/64/96 (max 128/32/64/32
partitions); matmul operands must start at partition 0. Misaligned
extractions (v_aux at rows 88:108) go through DMA.
"""

import numpy as np

B, N, C = 64, 236, 768
H, D = 8, 96
S1, S2 = 196, 216
BIAS = 100.0
SCALE = D ** -0.5
BIAS_RAW = BIAS / SCALE  # applied on raw (pre-scale) scores

N_CORES = 8
B_LOC = B // N_CORES

NT = [(0, 128), (128, 108)]  # token tiles (partition dim) / key tiles
NC_CH = C // 128  # 6 contraction chunks over C
KEXT = 128  # contraction size for scores: 96 head dims + bias rows
NW_QKV = 3 * C // 128  # 18 row-chunks of qkv_w
NW_PROJ = C // 128  # 6 row-chunks of proj_w
# head-aligned f-chunks for v and proj (psum bank = 512 f32)
FCH = [(0, 480), (480, 288)]  # heads 0:5 | 5:8 x 96


def part_cap(s):
    return 128 if s == 0 else 64 if s == 64 else 32


def part_pieces2(s1, s2, size):
    """Split a partition-range copy (dest start s1, src start s2, length
    size) into pieces legal for compute engines on both sides."""
    out = []
    off = 0
    while off < size:
        take = min(size - off, part_cap((s1 + off) % 128),
                   part_cap((s2 + off) % 128))
        out.append((s1 + off, s2 + off, take))
        off += take
    return out


def head_fragments(o_lo, o_hi, base):
    """Split channel range [o_lo, o_hi) (relative to `base`) at head
    boundaries (96) and legal partition pieces. Yields
    (head, d_lo, d_hi, p_lo, p_hi) with p relative to o_lo."""
    frags = []
    g = o_lo
    while g < o_hi:
        h = (g - base) // D
        d_lo = (g - base) - h * D
        take = min(o_hi - g, D - d_lo)
        for (d0, p0, sz) in part_pieces2(d_lo, g - o_lo, take):
            frags.append((h, d0, d0 + sz, p0, p0 + sz))
        g += take
    return frags


def build(b_loc=B_LOC):
    import concourse.bass as bass  # noqa: F401
    import concourse.tile as tile
    import concourse.bacc as bacc
    from concourse import mybir
    from concourse.masks import make_identity

    f32 = mybir.dt.float32
    bf16 = mybir.dt.bfloat16
    AF = mybir.ActivationFunctionType
    OP = mybir.AluOpType

    nc = bacc.Bacc("TRN2", target_bir_lowering=False)
    x_d = nc.dram_tensor("x", [b_loc, N, C], f32, kind="ExternalInput")
    qkvw_d = nc.dram_tensor("qkv_w", [3 * C, C], f32, kind="ExternalInput")
    projw_d = nc.dram_tensor("proj_w", [C, C], f32, kind="ExternalInput")
    projb_d = nc.dram_tensor("proj_b", [C], f32, kind="ExternalInput")
    out_d = nc.dram_tensor("out", [b_loc, N, C], f32, kind="ExternalOutput")

    with tile.TileContext(nc) as tc:
        with (
            tc.tile_pool(name="const", bufs=1) as constp,
            tc.tile_pool(name="wload", bufs=3) as wloadp,
            tc.tile_pool(name="xload", bufs=2) as xloadp,
            tc.tile_pool(name="xt", bufs=2) as xtp,
            tc.tile_pool(name="qk", bufs=2) as qkp,
            tc.tile_pool(name="vsb", bufs=2) as vsbp,
            tc.tile_pool(name="psb", bufs=3) as psbp,
            tc.tile_pool(name="ao", bufs=2) as aop,
            tc.tile_pool(name="osb", bufs=2) as osbp,
            tc.tile_pool(name="tiny", bufs=6) as tinyp,
            tc.tile_pool(name="ps_mm", bufs=3, space="PSUM") as ps_mm,
            tc.tile_pool(name="ps_s", bufs=3, space="PSUM") as ps_s,
            tc.tile_pool(name="ps_o", bufs=2, space="PSUM") as ps_o,
        ):
            # ---------------- constants ----------------
            ident = constp.tile([128, 128], bf16)
            make_identity(nc, ident[:])

            # Bias-extension master rows (contraction rows 96:128).
            # wmaster (q side): row0 = -BIAS_RAW on img+aux queries;
            #                   row1 = -BIAS_RAW on lang+aux queries.
            # umaster (k side): row0 = 1 on lang key slots;
            #                   row1 = 1 on aux key slots.
            wmaster = constp.tile([32, N], bf16)
            umaster = constp.tile([32, N], bf16)
            nc.vector.memset(wmaster[:], 0.0)
            nc.vector.memset(umaster[:], 0.0)
            nc.vector.memset(wmaster[0:1, 0:S1], -BIAS_RAW)
            nc.vector.memset(wmaster[0:1, S2:N], -BIAS_RAW)
            nc.vector.memset(umaster[0:1, S1:S2], 1.0)
            w2row = constp.tile([1, N], bf16)
            nc.vector.memset(w2row[:], 0.0)
            nc.vector.memset(w2row[0:1, S1:N], -BIAS_RAW)
            u2row = constp.tile([1, N], bf16)
            nc.vector.memset(u2row[:], 0.0)
            nc.vector.memset(u2row[0:1, S2:N], 1.0)
            nc.sync.dma_start(wmaster[1:2, :], w2row[:])
            nc.sync.dma_start(umaster[1:2, :], u2row[:])
            # per-head materialized copies (a stride-0 broadcast read on
            # DVE costs ~3us; 16 small contiguous copies cost ~2.4us
            # once, and the per-batch refill becomes a ~0.3us copy)
            wbig = constp.tile([32, H, N], bf16)
            ubig = constp.tile([32, H, N], bf16)
            for h in range(H):
                nc.vector.tensor_copy(wbig[:, h, :], wmaster[:])
                nc.vector.tensor_copy(ubig[:, h, :], umaster[:])

            # persistent weight tiles; proj head 7 row 96 = proj_b
            qkvwT = constp.tile([128, NC_CH, 3 * C], bf16)
            projwT = constp.tile([97, H, C], bf16)
            pb_row = constp.tile([1, C], f32)
            pb_bf = constp.tile([1, C], bf16)
            nc.sync.dma_start(pb_row[:], projb_d[None, :])
            nc.vector.tensor_copy(pb_bf[:], pb_row[:])
            nc.sync.dma_start(projwT[96:97, 7, :], pb_bf[:])

            # psum evacuations and casts alternate Vector/Scalar
            # (GpSimd cannot read PSUM and is ~6x slower on copies)
            cp_state = [0]

            def evac(dst, src):
                k = cp_state[0]
                cp_state[0] += 1
                if k % 2 == 1:
                    nc.scalar.copy(dst, src)
                else:
                    nc.vector.tensor_copy(dst, src)

            # ---------------- per-batch A-stage pieces ----------------
            st = {}  # b -> dict of tiles

            def a_xld(b):
                s = st[b] = {}
                s["xf"] = [xloadp.tile([128, C], f32, tag=f"xf{nt}",
                                       name=f"xf{nt}")
                           for nt in range(2)]
                for nt, (noff, nsz) in enumerate(NT):
                    nc.sync.dma_start(s["xf"][nt][:nsz],
                                      x_d[b, noff:noff + nsz, :])
                s["q_all"] = qkp.tile([KEXT, H, N], bf16, tag="q_all",
                                      name="q_all")
                s["k_all"] = qkp.tile([KEXT, H, N], bf16, tag="k_all",
                                      name="k_all")
                nc.vector.tensor_copy(s["q_all"][96:128, :, :], wbig[:])
                nc.vector.tensor_copy(s["k_all"][96:128, :, :], ubig[:])
                s["vp"] = [vsbp.tile([128, H, D + 1], bf16, tag=f"vp{nt}",
                                     name=f"vp{nt}")
                           for nt in range(2)]
                for nt, (noff, nsz) in enumerate(NT):
                    nc.vector.memset(s["vp"][nt][:nsz, :, D:D + 1], 1.0)
                s["xT"] = xtp.tile([128, NC_CH, N], bf16, tag="xT",
                                   name="xT")

            def a_xt(b, nt):
                """cast + transpose token tile nt into xT (paired
                transposes: 2 ci chunks per psum tile, halving evacs)."""
                s = st[b]
                noff, nsz = NT[nt]
                xb = xloadp.tile([128, C], bf16, tag=f"xb{nt}",
                                 name=f"xb{nt}")
                evac(xb[:nsz], s["xf"][nt][:nsz])
                for cp in range(NC_CH // 2):
                    pt = ps_mm.tile([128, 2, 128], bf16, tag="mm",
                                    name="mm")
                    for kk in range(2):
                        ci = 2 * cp + kk
                        nc.tensor.matmul(
                            pt[:, kk, :nsz],
                            xb[:nsz, ci * 128:(ci + 1) * 128],
                            ident[:nsz, :nsz], is_transpose=True,
                            skip_group_check=True)
                    evac(s["xT"][:, 2 * cp:2 * cp + 2, noff:noff + nsz],
                         pt[:, :, :nsz])

            def a_qk(b, oi):
                """q,k out-chunk oi (128 channels of q (oi<6) or k)."""
                s = st[b]
                ps = ps_mm.tile([128, N], f32, tag="mm", name="mm")
                for ci in range(NC_CH):
                    nc.tensor.matmul(
                        ps[:, :], qkvwT[:, ci, oi * 128:(oi + 1) * 128],
                        s["xT"][:, ci, :],
                        start=(ci == 0), stop=(ci == NC_CH - 1))
                t = (oi * 128) // C
                dst = s["q_all"] if t == 0 else s["k_all"]
                for (h, d_lo, d_hi, p_lo, p_hi) in head_fragments(
                        oi * 128, (oi + 1) * 128, t * C):
                    evac(dst[d_lo:d_hi, h, :], ps[p_lo:p_hi, :])

            def a_v(b, nt, fi):
                """v chunk: tokens tile nt, head-aligned channels fi,
                copied straight into the [n, h, d] layout."""
                s = st[b]
                noff, nsz = NT[nt]
                f0, fsz = FCH[fi]
                h0, nh = (0, 5) if fi == 0 else (5, 3)
                ps = ps_mm.tile([128, 480], f32, tag="mm", name="mm")
                for ci in range(NC_CH):
                    nc.tensor.matmul(
                        ps[:nsz, :fsz],
                        s["xT"][:, ci, noff:noff + nsz],
                        qkvwT[:, ci, 2 * C + f0:2 * C + f0 + fsz],
                        start=(ci == 0), stop=(ci == NC_CH - 1))
                evac(s["vp"][nt][:nsz, h0:h0 + nh, 0:D],
                     ps[:nsz, :fsz].rearrange("p (h d) -> p h d", d=D))

            def a_kx(b):
                """aux-key stash + pre-bias copy edit + v_aux extract."""
                s = st[b]
                s["k_aux"] = qkp.tile([96, H, S2 - S1], bf16, tag="k_aux",
                                      name="k_aux")
                nc.vector.tensor_copy(s["k_aux"][:, :, :],
                                      s["k_all"][0:96, :, S2:N])
                nc.vector.tensor_copy(s["k_all"][0:96, :, S2:N],
                                      s["k_all"][0:96, :, S1:S2])
                s["vap"] = vsbp.tile([S2 - S1, H, D + 1], bf16, tag="vap",
                                     name="vap")
                nc.sync.dma_start(s["vap"][:], s["vp"][1][88:108, :, :])

            # ---------------- per-batch B-stage pieces ----------------
            def b_sc(b, hp):
                """scores for head pair hp: 2 key tiles + aux, + exp."""
                s = st[b]
                h0 = 2 * hp
                s.setdefault("pe", {})
                for jt, (joff, jsz) in enumerate(NT):
                    psj = ps_s.tile([128, 2, N], f32, tag="s", name="s")
                    for hh in range(2):
                        nc.tensor.matmul(
                            psj[:jsz, hh, :],
                            s["k_all"][:, h0 + hh, joff:joff + jsz],
                            s["q_all"][:, h0 + hh, :], start=True,
                            stop=True, skip_group_check=True)
                    pe = psbp.tile([128, 2, N], bf16, tag="p", bufs=5,
                                   name="p")
                    nc.scalar.activation(pe[:jsz], psj[:jsz], AF.Exp,
                                         scale=SCALE)
                    s["pe"][(hp, jt)] = pe
                ps_aa = ps_s.tile([S2 - S1, 2, S2 - S1], f32, tag="s",
                                  name="s")
                for hh in range(2):
                    nc.tensor.matmul(ps_aa[:, hh, :],
                                     s["k_aux"][:, h0 + hh, :],
                                     s["q_all"][0:96, h0 + hh, S2:N],
                                     start=True, stop=True,
                                     skip_group_check=True)
                p_aa = tinyp.tile([S2 - S1, 2, S2 - S1], bf16, tag="paa",
                                  name="paa")
                nc.scalar.activation(p_aa[:], ps_aa[:], AF.Exp,
                                     scale=SCALE)
                s["pe"][(hp, "aa")] = p_aa

            def b_pso(b, hp):
                """attn @ [v|ones] for head pair hp + denom reciprocal
                broadcast (the aoT mult is deferred to b_mu)."""
                s = st[b]
                h0 = 2 * hp
                if "aoT" not in s:
                    s["aoT"] = aop.tile([97, H, N], bf16, tag="aoT",
                                        name="aoT")
                    nc.vector.memset(s["aoT"][96:97, :, :], 1.0)
                pso = ps_o.tile([D + 1, 2, N], f32, tag="o", name="o")
                for hh in range(2):
                    for jt, (joff, jsz) in enumerate(NT):
                        nc.tensor.matmul(pso[:, hh, :],
                                         s["vp"][jt][:jsz, h0 + hh, :],
                                         s["pe"][(hp, jt)][:jsz, hh, :],
                                         start=(jt == 0), stop=False,
                                         skip_group_check=True)
                    nc.tensor.matmul(pso[:, hh, S2:N],
                                     s["vap"][:, h0 + hh, :],
                                     s["pe"][(hp, "aa")][:, hh, :],
                                     start=False, stop=True,
                                     skip_group_check=True)
                den = tinyp.tile([1, 2, N], f32, tag="den", name="den")
                nc.vector.tensor_copy(den[:], pso[D:D + 1, :, :])
                r_f = tinyp.tile([1, 2, N], f32, tag="rf", name="rf")
                nc.vector.reciprocal_approx_fast(r_f[:], den[:])
                rbc = psbp.tile([96, 2, N], f32, tag="rbc", name="rbc")
                nc.gpsimd.partition_broadcast(
                    rbc[:], r_f[0:1, :, :].rearrange("p a b -> p (a b)"))
                s[("pso", hp)] = pso
                s[("rbc", hp)] = rbc

            def b_mu(b, hp):
                """normalize head pair hp into aoT (deferred mult)."""
                s = st[b]
                h0 = 2 * hp
                nc.vector.tensor_tensor(
                    s["aoT"][0:D, h0:h0 + 2, :], s[("pso", hp)][0:D, :, :],
                    s[("rbc", hp)][0:D, :, :], OP.mult)

            def b_pj(b, nt, fi):
                """proj chunk (tokens nt, channels fi) + bias + store."""
                s = st[b]
                noff, nsz = NT[nt]
                f0, fsz = FCH[fi]
                ps = ps_mm.tile([128, 480], f32, tag="mm", name="mm")
                for h in range(H):
                    kk = 97 if h == 7 else 96
                    nc.tensor.matmul(
                        ps[:nsz, :fsz],
                        s["aoT"][0:kk, h, noff:noff + nsz],
                        projwT[0:kk, h, f0:f0 + fsz],
                        start=(h == 0), stop=(h == H - 1))
                osb = osbp.tile([128, 480], f32, tag="osb", name="osb")
                evac(osb[:nsz, :fsz], ps[:nsz, :fsz])
                nc.sync.dma_start(out_d[b, noff:noff + nsz, f0:f0 + fsz],
                                  osb[:nsz, :fsz])

            # ---------------- weight prep pieces ----------------
            def w_row_qkv(r):
                wl = wloadp.tile([128, C], f32, tag="wl", name="wl")
                nc.sync.dma_start(wl[:], qkvw_d[r * 128:(r + 1) * 128, :])
                wb = wloadp.tile([128, C], bf16, tag="wb", name="wb")
                evac(wb[:], wl[:])
                for cp in range(NC_CH // 3):
                    pt = ps_mm.tile([128, 3, 128], bf16, tag="mm",
                                    name="mm")
                    for kk in range(3):
                        ci = 3 * cp + kk
                        nc.tensor.matmul(
                            pt[:, kk, :],
                            wb[:, ci * 128:(ci + 1) * 128], ident[:],
                            is_transpose=True, skip_group_check=True)
                    evac(qkvwT[:, 3 * cp:3 * cp + 3,
                               r * 128:(r + 1) * 128], pt[:])

            def w_row_proj(r):
                wl = wloadp.tile([128, C], f32, tag="wl", name="wl")
                nc.sync.dma_start(wl[:], projw_d[r * 128:(r + 1) * 128, :])
                wb = wloadp.tile([128, C], bf16, tag="wb", name="wb")
                evac(wb[:], wl[:])
                for hp, (hh0, nh) in enumerate([(0, 3), (3, 3), (6, 2)]):
                    pt = ps_mm.tile([128, 3, 128], bf16, tag="mm",
                                    name="mm")
                    for kk in range(nh):
                        h = hh0 + kk
                        nc.tensor.matmul(
                            pt[:96, kk, :], wb[:, h * D:(h + 1) * D],
                            ident[:], is_transpose=True,
                            skip_group_check=True)
                    evac(projwT[0:96, hh0:hh0 + nh,
                                r * 128:(r + 1) * 128],
                         pt[:96, :nh, :])

            # ---------------- emission schedule ----------------
            # prep + A(0): pipeline qkv_w rows with batch 0's qkv
            for r in range(NW_QKV):
                w_row_qkv(r)
                if r == 0:
                    a_xld(0)
                if r == 1:
                    a_xt(0, 0)
                    a_xt(0, 1)
                if 3 <= r <= 14:
                    a_qk(0, r - 3)
            for nt in range(2):
                for fi in range(2):
                    a_v(0, nt, fi)
            for r in range(NW_PROJ):
                w_row_proj(r)
            a_kx(0)

            def interleave(bp, ap):
                """Emit B(b) pieces with A(b+1) pieces between them."""
                sched = [
                    ("B", 0), ("A", 0), ("B", 1), ("A", 1), ("A", 2),
                    ("B", 2), ("A", 3), ("B", 3), ("A", 4), ("B", 4),
                    ("B", 5), ("A", 5), ("B", 6), ("A", 6), ("B", 7),
                    ("B", 8), ("A", 7), ("A", 8), ("B", 9), ("B", 10),
                    ("A", 9), ("A", 10), ("B", 11), ("A", 11), ("B", 12),
                    ("A", 12), ("B", 13), ("A", 13), ("B", 14), ("A", 14),
                    ("B", 15), ("A", 15), ("A", 16), ("A", 17), ("A", 18),
                    ("A", 19),
                ]
                for kind, i in sched:
                    lst = bp if kind == "B" else ap
                    if i < len(lst):
                        lst[i]()

            for b in range(b_loc):
                bp = [
                    lambda b=b: b_sc(b, 0),
                    lambda b=b: b_sc(b, 1),
                    lambda b=b: b_pso(b, 0),
                    lambda b=b: b_sc(b, 2),
                    lambda b=b: b_pso(b, 1),
                    lambda b=b: b_mu(b, 0),
                    lambda b=b: b_sc(b, 3),
                    lambda b=b: b_pso(b, 2),
                    lambda b=b: b_mu(b, 1),
                    lambda b=b: b_pso(b, 3),
                    lambda b=b: b_mu(b, 2),
                    lambda b=b: b_mu(b, 3),
                    lambda b=b: b_pj(b, 0, 0),
                    lambda b=b: b_pj(b, 0, 1),
                    lambda b=b: b_pj(b, 1, 0),
                    lambda b=b: b_pj(b, 1, 1),
                ]
                ap = []
                if b + 1 < b_loc:
                    bn = b + 1
                    ap = [
                        lambda bn=bn: a_xld(bn),
                        lambda bn=bn: a_xt(bn, 0),
                        lambda bn=bn: a_xt(bn, 1),
                    ] + [
                        lambda bn=bn, oi=oi: a_qk(bn, oi)
                        for oi in range(12)
                    ] + [
                        lambda bn=bn, nt=nt, fi=fi: a_v(bn, nt, fi)
                        for nt in range(2) for fi in range(2)
                    ] + [lambda bn=bn: a_kx(bn)]
                interleave(bp, ap)
                st.pop(b, None)

    nc.compile()
    return nc


_NC_CACHE = {}


def _get_nc(b_loc):
    if b_loc not in _NC_CACHE:
        _NC_CACHE[b_loc] = build(b_loc)
    return _NC_CACHE[b_loc]


def _run(inputs, trace=False):
    from concourse.bass_utils import run_bass_kernel_spmd

    x = np.ascontiguousarray(np.asarray(inputs["x"], dtype=np.float32))
    qkv_w = np.ascontiguousarray(np.asarray(inputs["qkv_w"],
                                            dtype=np.float32))
    proj_w = np.ascontiguousarray(np.asarray(inputs["proj_w"],
                                             dtype=np.float32))
    proj_b = np.ascontiguousarray(np.asarray(inputs["proj_b"],
                                             dtype=np.float32))

    nc = _get_nc(B_LOC)
    in_maps = [
        {
            "x": np.ascontiguousarray(x[i * B_LOC:(i + 1) * B_LOC]),
            "qkv_w": qkv_w,
            "proj_w": proj_w,
            "proj_b": proj_b,
        }
        for i in range(N_CORES)
    ]
    res = run_bass_kernel_spmd(
        nc, in_maps, core_ids=list(range(N_CORES)), trace=trace)
    out = np.concatenate([r["out"] for r in res.results], axis=0)
    return out, res


def kernel(x, qkv_w, proj_w, proj_b):
    out, _ = _run({"x": x, "qkv_w": qkv_w, "proj_w": proj_w,
                   "proj_b": proj_b})
    return out


# revision 10
# speedup vs baseline: 1.2196x; 1.2196x over previous
"""Trainium2 Bass kernel for the sparse_attention PoC block.

Reference computation (per batch item):
  qkv = x @ qkv_w.T            [N, 3C] -> q,k,v heads [H, N, D]
  attn = (q @ k.T) * scale     [H, N, N]
  block edits: attn[:S1, S2:] = attn[:S1, S1:S2] (pre-bias copy), then
  -100 bias on [:S1, S1:S2], [S1:S2, S2:], [S2:, S1:S2]; softmax;
  attn @ v; proj.

Distribution: pure data-parallel over batch B=64 across 8 NeuronCores
(8 batch items per core, weights replicated). No collectives.

v3: software-pipelined emission. The PE (TensorMatrix) is the
bottleneck (~400k row-cycles ~= 167us at the 2.4 GHz max p-state); the
baseline lost ~2x to dependency stalls in the attention phase (PE idle
gaps also drop the DVFS p-state to 1.2 GHz). Batch b's attention+proj
pieces are interleaved at EMISSION level with batch b+1's
qkv-projection pieces so the in-order PE queue always has independent
matmuls between dependent ones; weight prep is likewise pipelined with
batch 0's qkv. Engine assignment: ScalarE+VectorE alternate on psum
evacuations and casts; GpSimd only does the reciprocal partition
broadcasts (it cannot read PSUM and its copy rate is ~6x worse than
V/S); per-batch q/k bias-extension rows are copied from pre-broadcast
wbig/ubig tiles (a stride-0 broadcast read costs 3us vs 0.3us
contiguous). Transposes are paired/tripled per psum tile to halve
evacuation instruction count. proj_b rides the proj matmul as a 97th
contraction row on head 7 (aoT row 96 = ones).

Layout per core (TensorE matmuls in bf16, fp32 PSUM accum):
  - x transposed on TensorE to xT [C, N]
  - q,k per head transposed: q_all/k_all [128, H, N] (rows 0:96 head
    data, 96:128 bias-extension rows); v natural [n, h, D+1] ([v|ones])
  - scores sT[j,q] = k_ext.T @ q_ext in one matmul per key tile; the
    -100 block edits ride as rank-2 updates in contraction rows 96/97;
    the pre-bias "copy" edit = overwriting kT aux-slot columns with
    lang key vectors + a 20x20 correction matmul for the aux x aux
    block
  - softmax without max-subtraction (logits O(1); suppressed entries
    underflow exp to ~0); exp on ScalarE with 1/sqrt(D) folded in
  - attn@v gives oT [D+1, q] + denominator row; normalize via approx
    reciprocal + gpsimd partition_broadcast (96 rows); the aoT mult is
    emitted 2 schedule slots after its broadcast so the in-order
    Vector queue never waits on GpSimd
  - proj psum[n, oc] = aoT.T @ proj_w^T (K=96/97 per head) -> osb ->
    DRAM (PSUM cannot be a DMA source)

Known HW constraint: reciprocal_approx_fast with a PSUM source dies at
runtime (isolated probe passes, in-kernel use does not) -- the
denominator is staged through SBUF first.

Partition-alignment rule (walrus verifier): compute-engine access
patterns must start at partition 0/32/64/96 (max 128/32/64/32
partitions); matmul operands must start at partition 0. Misaligned
extractions (v_aux at rows 88:108) go through DMA.
"""

import numpy as np

B, N, C = 64, 236, 768
H, D = 8, 96
S1, S2 = 196, 216
BIAS = 100.0
SCALE = D ** -0.5
BIAS_RAW = BIAS / SCALE  # applied on raw (pre-scale) scores

N_CORES = 8
B_LOC = B // N_CORES

NT = [(0, 128), (128, 108)]  # token tiles (partition dim) / key tiles
NC_CH = C // 128  # 6 contraction chunks over C
KEXT = 128  # contraction size for scores: 96 head dims + bias rows
NW_QKV = 3 * C // 128  # 18 row-chunks of qkv_w
NW_PROJ = C // 128  # 6 row-chunks of proj_w
# head-aligned f-chunks for v and proj (psum bank = 512 f32)
FCH = [(0, 480), (480, 288)]  # heads 0:5 | 5:8 x 96


def part_cap(s):
    return 128 if s == 0 else 64 if s == 64 else 32


def part_pieces2(s1, s2, size):
    """Split a partition-range copy (dest start s1, src start s2, length
    size) into pieces legal for compute engines on both sides."""
    out = []
    off = 0
    while off < size:
        take = min(size - off, part_cap((s1 + off) % 128),
                   part_cap((s2 + off) % 128))
        out.append((s1 + off, s2 + off, take))
        off += take
    return out


def head_fragments(o_lo, o_hi, base):
    """Split channel range [o_lo, o_hi) (relative to `base`) at head
    boundaries (96) and legal partition pieces. Yields
    (head, d_lo, d_hi, p_lo, p_hi) with p relative to o_lo."""
    frags = []
    g = o_lo
    while g < o_hi:
        h = (g - base) // D
        d_lo = (g - base) - h * D
        take = min(o_hi - g, D - d_lo)
        for (d0, p0, sz) in part_pieces2(d_lo, g - o_lo, take):
            frags.append((h, d0, d0 + sz, p0, p0 + sz))
        g += take
    return frags


def build(b_loc=B_LOC):
    import concourse.bass as bass  # noqa: F401
    import concourse.tile as tile
    import concourse.bacc as bacc
    from concourse import mybir
    from concourse.masks import make_identity

    f32 = mybir.dt.float32
    bf16 = mybir.dt.bfloat16
    AF = mybir.ActivationFunctionType
    OP = mybir.AluOpType

    nc = bacc.Bacc("TRN2", target_bir_lowering=False)
    x_d = nc.dram_tensor("x", [b_loc, N, C], f32, kind="ExternalInput")
    qkvw_d = nc.dram_tensor("qkv_w", [3 * C, C], f32, kind="ExternalInput")
    projw_d = nc.dram_tensor("proj_w", [C, C], f32, kind="ExternalInput")
    projb_d = nc.dram_tensor("proj_b", [C], f32, kind="ExternalInput")
    out_d = nc.dram_tensor("out", [b_loc, N, C], f32, kind="ExternalOutput")

    with tile.TileContext(nc) as tc:
        with (
            tc.tile_pool(name="const", bufs=1) as constp,
            tc.tile_pool(name="wload", bufs=3) as wloadp,
            tc.tile_pool(name="xload", bufs=2) as xloadp,
            tc.tile_pool(name="xt", bufs=2) as xtp,
            tc.tile_pool(name="qk", bufs=2) as qkp,
            tc.tile_pool(name="vsb", bufs=2) as vsbp,
            tc.tile_pool(name="psb", bufs=3) as psbp,
            tc.tile_pool(name="ao", bufs=2) as aop,
            tc.tile_pool(name="osb", bufs=2) as osbp,
            tc.tile_pool(name="tiny", bufs=6) as tinyp,
            tc.tile_pool(name="ps_mm", bufs=4, space="PSUM") as ps_mm,
            tc.tile_pool(name="ps_s", bufs=2, space="PSUM") as ps_s,
            tc.tile_pool(name="ps_o", bufs=2, space="PSUM") as ps_o,
        ):
            # ---------------- constants ----------------
            ident = constp.tile([128, 128], bf16)
            make_identity(nc, ident[:])

            # Bias-extension master rows (contraction rows 96:128).
            # wmaster (q side): row0 = -BIAS_RAW on img+aux queries;
            #                   row1 = -BIAS_RAW on lang+aux queries.
            # umaster (k side): row0 = 1 on lang key slots;
            #                   row1 = 1 on aux key slots.
            wmaster = constp.tile([32, N], bf16)
            umaster = constp.tile([32, N], bf16)
            nc.vector.memset(wmaster[:], 0.0)
            nc.vector.memset(umaster[:], 0.0)
            nc.vector.memset(wmaster[0:1, 0:S1], -BIAS_RAW)
            nc.vector.memset(wmaster[0:1, S2:N], -BIAS_RAW)
            nc.vector.memset(umaster[0:1, S1:S2], 1.0)
            w2row = constp.tile([1, N], bf16)
            nc.vector.memset(w2row[:], 0.0)
            nc.vector.memset(w2row[0:1, S1:N], -BIAS_RAW)
            u2row = constp.tile([1, N], bf16)
            nc.vector.memset(u2row[:], 0.0)
            nc.vector.memset(u2row[0:1, S2:N], 1.0)
            nc.sync.dma_start(wmaster[1:2, :], w2row[:])
            nc.sync.dma_start(umaster[1:2, :], u2row[:])
            # per-head materialized copies (a stride-0 broadcast read on
            # DVE costs ~3us; 16 small contiguous copies cost ~2.4us
            # once, and the per-batch refill becomes a ~0.3us copy).
            # Built after w_row_qkv(0) so they don't delay the first
            # weight cast in the Vector queue.
            wbig = constp.tile([32, H, N], bf16)
            ubig = constp.tile([32, H, N], bf16)

            def build_wubig():
                for h in range(H):
                    nc.vector.tensor_copy(wbig[:, h, :], wmaster[:])
                    nc.vector.tensor_copy(ubig[:, h, :], umaster[:])

            # persistent weight tiles; proj head 7 row 96 = proj_b
            qkvwT = constp.tile([128, NC_CH, 3 * C], bf16)
            projwT = constp.tile([97, H, C], bf16)
            pb_row = constp.tile([1, C], f32)
            pb_bf = constp.tile([1, C], bf16)
            nc.sync.dma_start(pb_row[:], projb_d[None, :])
            nc.vector.tensor_copy(pb_bf[:], pb_row[:])
            nc.sync.dma_start(projwT[96:97, 7, :], pb_bf[:])

            # psum evacuations and casts alternate Vector/Scalar
            # (GpSimd cannot read PSUM and is ~6x slower on copies)
            cp_state = [0]

            def evac(dst, src):
                k = cp_state[0]
                cp_state[0] += 1
                if k % 2 == 1:
                    nc.scalar.copy(dst, src)
                else:
                    nc.vector.tensor_copy(dst, src)

            # ---------------- per-batch A-stage pieces ----------------
            st = {}  # b -> dict of tiles

            def a_xld(b):
                s = st[b] = {}
                s["xf"] = [xloadp.tile([128, C], f32, tag=f"xf{nt}",
                                       name=f"xf{nt}")
                           for nt in range(2)]
                for nt, (noff, nsz) in enumerate(NT):
                    nc.sync.dma_start(s["xf"][nt][:nsz],
                                      x_d[b, noff:noff + nsz, :])
                s["q_all"] = qkp.tile([KEXT, H, N], bf16, tag="q_all",
                                      name="q_all")
                s["k_all"] = qkp.tile([KEXT, H, N], bf16, tag="k_all",
                                      name="k_all")
                nc.vector.tensor_copy(s["q_all"][96:128, :, :], wbig[:])
                nc.vector.tensor_copy(s["k_all"][96:128, :, :], ubig[:])
                s["vp"] = [vsbp.tile([128, H, D + 1], bf16, tag=f"vp{nt}",
                                     name=f"vp{nt}")
                           for nt in range(2)]
                for nt, (noff, nsz) in enumerate(NT):
                    nc.vector.memset(s["vp"][nt][:nsz, :, D:D + 1], 1.0)
                s["xT"] = xtp.tile([128, NC_CH, N], bf16, tag="xT",
                                   name="xT")

            def a_xt(b, nt):
                """cast + transpose token tile nt into xT (paired
                transposes: 2 ci chunks per psum tile, halving evacs)."""
                s = st[b]
                noff, nsz = NT[nt]
                xb = xloadp.tile([128, C], bf16, tag=f"xb{nt}",
                                 name=f"xb{nt}")
                evac(xb[:nsz], s["xf"][nt][:nsz])
                for cp in range(NC_CH // 2):
                    pt = ps_mm.tile([128, 2, 128], bf16, tag="mm",
                                    name="mm")
                    for kk in range(2):
                        ci = 2 * cp + kk
                        nc.tensor.matmul(
                            pt[:, kk, :nsz],
                            xb[:nsz, ci * 128:(ci + 1) * 128],
                            ident[:nsz, :nsz], is_transpose=True,
                            skip_group_check=True)
                    evac(s["xT"][:, 2 * cp:2 * cp + 2, noff:noff + nsz],
                         pt[:, :, :nsz])

            def a_qk(b, oi):
                """q,k out-chunk oi (128 channels of q (oi<6) or k)."""
                s = st[b]
                ps = ps_mm.tile([128, N], f32, tag="mm", name="mm")
                for ci in range(NC_CH):
                    nc.tensor.matmul(
                        ps[:, :], qkvwT[:, ci, oi * 128:(oi + 1) * 128],
                        s["xT"][:, ci, :],
                        start=(ci == 0), stop=(ci == NC_CH - 1))
                t = (oi * 128) // C
                dst = s["q_all"] if t == 0 else s["k_all"]
                for (h, d_lo, d_hi, p_lo, p_hi) in head_fragments(
                        oi * 128, (oi + 1) * 128, t * C):
                    evac(dst[d_lo:d_hi, h, :], ps[p_lo:p_hi, :])

            def a_v(b, nt, fi):
                """v chunk: tokens tile nt, head-aligned channels fi,
                copied straight into the [n, h, d] layout."""
                s = st[b]
                noff, nsz = NT[nt]
                f0, fsz = FCH[fi]
                h0, nh = (0, 5) if fi == 0 else (5, 3)
                ps = ps_mm.tile([128, 480], f32, tag="mm", name="mm")
                for ci in range(NC_CH):
                    nc.tensor.matmul(
                        ps[:nsz, :fsz],
                        s["xT"][:, ci, noff:noff + nsz],
                        qkvwT[:, ci, 2 * C + f0:2 * C + f0 + fsz],
                        start=(ci == 0), stop=(ci == NC_CH - 1))
                evac(s["vp"][nt][:nsz, h0:h0 + nh, 0:D],
                     ps[:nsz, :fsz].rearrange("p (h d) -> p h d", d=D))

            def a_kx(b):
                """aux-key stash + pre-bias copy edit + v_aux extract."""
                s = st[b]
                s["k_aux"] = qkp.tile([96, H, S2 - S1], bf16, tag="k_aux",
                                      name="k_aux")
                nc.vector.tensor_copy(s["k_aux"][:, :, :],
                                      s["k_all"][0:96, :, S2:N])
                nc.vector.tensor_copy(s["k_all"][0:96, :, S2:N],
                                      s["k_all"][0:96, :, S1:S2])
                s["vap"] = vsbp.tile([S2 - S1, H, D + 1], bf16, tag="vap",
                                     name="vap")
                nc.sync.dma_start(s["vap"][:], s["vp"][1][88:108, :, :])

            # ---------------- per-batch B-stage pieces ----------------
            def b_sc(b, hp):
                """scores for head pair hp: 2 key tiles + aux, + exp."""
                s = st[b]
                h0 = 2 * hp
                s.setdefault("pe", {})
                for jt, (joff, jsz) in enumerate(NT):
                    psj = ps_s.tile([128, 2, N], f32, tag="s", name="s")
                    for hh in range(2):
                        nc.tensor.matmul(
                            psj[:jsz, hh, :],
                            s["k_all"][:, h0 + hh, joff:joff + jsz],
                            s["q_all"][:, h0 + hh, :], start=True,
                            stop=True, skip_group_check=True)
                    pe = psbp.tile([128, 2, N], bf16, tag="p", bufs=5,
                                   name="p")
                    nc.scalar.activation(pe[:jsz], psj[:jsz], AF.Exp,
                                         scale=SCALE)
                    s["pe"][(hp, jt)] = pe
                ps_aa = ps_s.tile([S2 - S1, 2, S2 - S1], f32, tag="s",
                                  name="s")
                for hh in range(2):
                    nc.tensor.matmul(ps_aa[:, hh, :],
                                     s["k_aux"][:, h0 + hh, :],
                                     s["q_all"][0:96, h0 + hh, S2:N],
                                     start=True, stop=True,
                                     skip_group_check=True)
                p_aa = tinyp.tile([S2 - S1, 2, S2 - S1], bf16, tag="paa",
                                  name="paa")
                nc.scalar.activation(p_aa[:], ps_aa[:], AF.Exp,
                                     scale=SCALE)
                s["pe"][(hp, "aa")] = p_aa

            def b_pso(b, hp):
                """attn @ [v|ones] for head pair hp + denom reciprocal
                broadcast (the aoT mult is deferred to b_mu)."""
                s = st[b]
                h0 = 2 * hp
                if "aoT" not in s:
                    s["aoT"] = aop.tile([97, H, N], bf16, tag="aoT",
                                        name="aoT")
                    nc.vector.memset(s["aoT"][96:97, :, :], 1.0)
                pso = ps_o.tile([D + 1, 2, N], f32, tag="o", name="o")
                for hh in range(2):
                    for jt, (joff, jsz) in enumerate(NT):
                        nc.tensor.matmul(pso[:, hh, :],
                                         s["vp"][jt][:jsz, h0 + hh, :],
                                         s["pe"][(hp, jt)][:jsz, hh, :],
                                         start=(jt == 0), stop=False,
                                         skip_group_check=True)
                    nc.tensor.matmul(pso[:, hh, S2:N],
                                     s["vap"][:, h0 + hh, :],
                                     s["pe"][(hp, "aa")][:, hh, :],
                                     start=False, stop=True,
                                     skip_group_check=True)
                den = tinyp.tile([1, 2, N], f32, tag="den", name="den")
                nc.vector.tensor_copy(den[:], pso[D:D + 1, :, :])
                r_f = tinyp.tile([1, 2, N], f32, tag="rf", name="rf")
                nc.vector.reciprocal_approx_fast(r_f[:], den[:])
                rbc = psbp.tile([96, 2, N], f32, tag="rbc", name="rbc")
                nc.gpsimd.partition_broadcast(
                    rbc[:], r_f[0:1, :, :].rearrange("p a b -> p (a b)"))
                s[("pso", hp)] = pso
                s[("rbc", hp)] = rbc

            def b_mu(b, hp):
                """normalize head pair hp into aoT (deferred mult)."""
                s = st[b]
                h0 = 2 * hp
                nc.vector.tensor_tensor(
                    s["aoT"][0:D, h0:h0 + 2, :], s[("pso", hp)][0:D, :, :],
                    s[("rbc", hp)][0:D, :, :], OP.mult)

            def b_pj(b, nt, fi):
                """proj chunk (tokens nt, channels fi) + bias + store."""
                s = st[b]
                noff, nsz = NT[nt]
                f0, fsz = FCH[fi]
                ps = ps_mm.tile([128, 480], f32, tag="mm", name="mm")
                for h in range(H):
                    kk = 97 if h == 7 else 96
                    nc.tensor.matmul(
                        ps[:nsz, :fsz],
                        s["aoT"][0:kk, h, noff:noff + nsz],
                        projwT[0:kk, h, f0:f0 + fsz],
                        start=(h == 0), stop=(h == H - 1))
                osb = osbp.tile([128, 480], f32, tag="osb", name="osb")
                evac(osb[:nsz, :fsz], ps[:nsz, :fsz])
                nc.sync.dma_start(out_d[b, noff:noff + nsz, f0:f0 + fsz],
                                  osb[:nsz, :fsz])

            # ---------------- weight prep pieces ----------------
            def w_row_qkv(r):
                wl = wloadp.tile([128, C], f32, tag="wl", name="wl")
                nc.sync.dma_start(wl[:], qkvw_d[r * 128:(r + 1) * 128, :])
                wb = wloadp.tile([128, C], bf16, tag="wb", name="wb")
                evac(wb[:], wl[:])
                for cp in range(NC_CH // 3):
                    pt = ps_mm.tile([128, 3, 128], bf16, tag="mm",
                                    name="mm")
                    for kk in range(3):
                        ci = 3 * cp + kk
                        nc.tensor.matmul(
                            pt[:, kk, :],
                            wb[:, ci * 128:(ci + 1) * 128], ident[:],
                            is_transpose=True, skip_group_check=True)
                    evac(qkvwT[:, 3 * cp:3 * cp + 3,
                               r * 128:(r + 1) * 128], pt[:])

            def w_row_proj(r):
                wl = wloadp.tile([128, C], f32, tag="wl", name="wl")
                nc.sync.dma_start(wl[:], projw_d[r * 128:(r + 1) * 128, :])
                wb = wloadp.tile([128, C], bf16, tag="wb", name="wb")
                evac(wb[:], wl[:])
                for hp, (hh0, nh) in enumerate([(0, 3), (3, 3), (6, 2)]):
                    pt = ps_mm.tile([128, 3, 128], bf16, tag="mm",
                                    name="mm")
                    for kk in range(nh):
                        h = hh0 + kk
                        nc.tensor.matmul(
                            pt[:96, kk, :], wb[:, h * D:(h + 1) * D],
                            ident[:], is_transpose=True,
                            skip_group_check=True)
                    evac(projwT[0:96, hh0:hh0 + nh,
                                r * 128:(r + 1) * 128],
                         pt[:96, :nh, :])

            # ---------------- emission schedule ----------------
            # prep + A(0): pipeline qkv_w rows with batch 0's qkv
            for r in range(NW_QKV):
                w_row_qkv(r)
                if r == 0:
                    build_wubig()
                    a_xld(0)
                if r == 1:
                    a_xt(0, 0)
                    a_xt(0, 1)
                if 3 <= r <= 14:
                    a_qk(0, r - 3)
            for nt in range(2):
                for fi in range(2):
                    a_v(0, nt, fi)
            for r in range(NW_PROJ):
                w_row_proj(r)
            a_kx(0)

            def interleave(bp, ap):
                """Emit B(b) pieces with A(b+1) pieces between them."""
                sched = [
                    ("B", 0), ("A", 0), ("B", 1), ("A", 1), ("A", 2),
                    ("B", 2), ("A", 3), ("B", 3), ("A", 4), ("B", 4),
                    ("B", 5), ("A", 5), ("B", 6), ("A", 6), ("B", 7),
                    ("B", 8), ("A", 7), ("A", 8), ("B", 9), ("B", 10),
                    ("A", 9), ("A", 10), ("B", 11), ("A", 11), ("B", 12),
                    ("A", 12), ("B", 13), ("A", 13), ("B", 14), ("A", 14),
                    ("B", 15), ("A", 15), ("A", 16), ("A", 17), ("A", 18),
                    ("A", 19),
                ]
                for kind, i in sched:
                    lst = bp if kind == "B" else ap
                    if i < len(lst):
                        lst[i]()

            for b in range(b_loc):
                bp = [
                    lambda b=b: b_sc(b, 0),
                    lambda b=b: b_sc(b, 1),
                    lambda b=b: b_pso(b, 0),
                    lambda b=b: b_sc(b, 2),
                    lambda b=b: b_pso(b, 1),
                    lambda b=b: b_mu(b, 0),
                    lambda b=b: b_sc(b, 3),
                    lambda b=b: b_pso(b, 2),
                    lambda b=b: b_mu(b, 1),
                    lambda b=b: b_pso(b, 3),
                    lambda b=b: b_mu(b, 2),
                    lambda b=b: b_mu(b, 3),
                    lambda b=b: b_pj(b, 0, 0),
                    lambda b=b: b_pj(b, 0, 1),
                    lambda b=b: b_pj(b, 1, 0),
                    lambda b=b: b_pj(b, 1, 1),
                ]
                ap = []
                if b + 1 < b_loc:
                    bn = b + 1
                    ap = [
                        lambda bn=bn: a_xld(bn),
                        lambda bn=bn: a_xt(bn, 0),
                        lambda bn=bn: a_xt(bn, 1),
                    ] + [
                        lambda bn=bn, oi=oi: a_qk(bn, oi)
                        for oi in range(12)
                    ] + [
                        lambda bn=bn, nt=nt, fi=fi: a_v(bn, nt, fi)
                        for nt in range(2) for fi in range(2)
                    ] + [lambda bn=bn: a_kx(bn)]
                interleave(bp, ap)
                st.pop(b, None)

    nc.compile()
    return nc


_NC_CACHE = {}


def _get_nc(b_loc):
    if b_loc not in _NC_CACHE:
        _NC_CACHE[b_loc] = build(b_loc)
    return _NC_CACHE[b_loc]


def _run(inputs, trace=False):
    from concourse.bass_utils import run_bass_kernel_spmd

    x = np.ascontiguousarray(np.asarray(inputs["x"], dtype=np.float32))
    qkv_w = np.ascontiguousarray(np.asarray(inputs["qkv_w"],
                                            dtype=np.float32))
    proj_w = np.ascontiguousarray(np.asarray(inputs["proj_w"],
                                             dtype=np.float32))
    proj_b = np.ascontiguousarray(np.asarray(inputs["proj_b"],
                                             dtype=np.float32))

    nc = _get_nc(B_LOC)
    in_maps = [
        {
            "x": np.ascontiguousarray(x[i * B_LOC:(i + 1) * B_LOC]),
            "qkv_w": qkv_w,
            "proj_w": proj_w,
            "proj_b": proj_b,
        }
        for i in range(N_CORES)
    ]
    res = run_bass_kernel_spmd(
        nc, in_maps, core_ids=list(range(N_CORES)), trace=trace)
    out = np.concatenate([r["out"] for r in res.results], axis=0)
    return out, res


def kernel(x, qkv_w, proj_w, proj_b):
    out, _ = _run({"x": x, "qkv_w": qkv_w, "proj_w": proj_w,
                   "proj_b": proj_b})
    return out
